# revision 49
# baseline (speedup 1.0000x reference)
"""Trainium2 Bass kernel for nn_Alpha_Cos_GLM.

Pipeline (two NEFF launches):
  Launch A (8 cores, data-parallel over time):
    syn_e = S_e @ C_syn_e.T, syn_i = S_i @ C_syn_i.T   (PE, bf16 -- exact for 0/1 data)
    syn_in = causal_conv(syn_e, ke) + causal_conv(syn_i, ki)
      -- depthwise 200-tap conv as per-subunit Toeplitz matmuls in a
         128-time-chunk layout (3 chunk-shift terms cover lags 0..200).
         Weights are bf16 hi+lo pairs (full fp32 precision, 1-pass bf16 rate).
  Launch B (1 core): the sequential scan
      y_t = tanh(syn_in_t + Theta + hist_conv(y) + tree_prop(y_{t-1}))
    via Jacobi fixed-point iteration over the whole padded sequence
    (contraction ~0.7/iter).  Iteration schedule: N_BF plain-bf16 iterations
    (cheap) followed by N_TAIL full-precision iterations where both the
    weights and the iterate are split into bf16 hi+lo parts (3 matmuls per
    logical matmul).  PSUM is preloaded with syn_in by DVE copies; the
    matmuls accumulate on top; one tanh ACT per PSUM bank.

Everything uses an s-major on-chip layout [subunit, chunk] so every matmul
rhs is a contiguous run.
"""

import os
import numpy as np
import ml_dtypes

import concourse.bass as bass
import concourse.tile as tile
from concourse import bacc, mybir
from concourse.bass_utils import run_bass_kernel_spmd
from contextlib import ExitStack

# ---------------- problem constants (hardcoded shapes) ----------------
T = 20000
E_NO = 2000
I_NO = 500
SUB = 20
T_NO = 200
NCORES = 8

CH = 128                 # time chunk
CPC = 20                 # owned chunks per core
NCHUNK = NCORES * CPC    # 160 global chunks
TPAD = NCHUNK * CH       # 20480
HALO = 2                 # halo chunks (lags up to 200 < 2*128)
LOCAL_CHUNKS = CPC + HALO
TLOC = LOCAL_CHUNKS * CH # 2816
YC = NCHUNK + HALO       # per-subunit columns in the scan y buffers

N_BF = int(os.environ.get("GLM_N_BF", "14"))
N_TAIL = int(os.environ.get("GLM_N_TAIL", "2"))

F32 = mybir.dt.float32
BF16 = mybir.dt.bfloat16
BF16_NP = ml_dtypes.bfloat16
FP8 = mybir.dt.float8e4
FP8_NP = ml_dtypes.float8_e4m3

E_TILES = [(o, min(128, E_NO - o)) for o in range(0, E_NO, 128)]
I_TILES = [(o, min(128, I_NO - o)) for o in range(0, I_NO, 128)]
COL_GROUPS = [(0, 11), (11, 11)]   # (chunk0, nchunks) DMA groups

# scan subunit groups -> one PSUM bank each (ns*NCHUNK floats <= 512)
GROUPS = [(0, 3), (3, 3), (6, 3), (9, 3), (12, 3), (15, 3), (18, 2)]

LAST_PROFILE = {}
_KCACHE = {}


def _maybe_trace():
    if not os.environ.get("GLM_TRACE"):
        return False
    try:  # enable NTFF profiling under axon; harmless no-op if unavailable
        import sys, types
        if "antenv.axon_hooks" not in sys.modules:
            mod = types.ModuleType("antenv.axon_hooks")
            mod._hook = None
            mod.set_axon_ntff_profile_hook = lambda h: setattr(mod, "_hook", h)
            mod.get_axon_ntff_profile_hook = lambda: mod._hook
            sys.modules["antenv.axon_hooks"] = mod
            import antenv
            antenv.axon_hooks = mod
            from trn_agent_boot.trn_boot import _ntff_profile_via_ctypes
            mod.set_axon_ntff_profile_hook(
                _ntff_profile_via_ctypes("/opt/axon/libaxon_pjrt.so"))
        return True
    except Exception:
        return False


# ---------------- host-side parameter math ----------------

def _alpha_kernels(W_syn, Tau_syn, Delta_syn):
    t = np.arange(T_NO, dtype=np.float32)
    te = np.maximum(t[None, None, :] - Delta_syn[:, :, 0, None], 0.0)
    ti = np.maximum(t[None, None, :] - Delta_syn[:, :, 1, None], 0.0)
    te = te / np.exp(Tau_syn[:, :, 0])[:, :, None]
    ti = ti / np.exp(Tau_syn[:, :, 1])[:, :, None]
    ke = np.sum(te * np.exp(-te) * W_syn[:, :, 0, None], axis=1)
    ki = np.sum(ti * np.exp(-ti) * W_syn[:, :, 1, None], axis=1)
    return ke.astype(np.float32), ki.astype(np.float32)


def _toeplitz_triple(wfun):
    """K_d[j,i] = w(i-j+128*d), d=0,1,2: out chunk c reads chunks c,c-1,c-2."""
    idx = np.arange(CH)
    D = idx[None, :] - idx[:, None]  # i - j
    return (wfun(D).astype(np.float32), wfun(D + CH).astype(np.float32),
            wfun(D + 2 * CH).astype(np.float32))


def _lagw(kern_row, lag0):
    """w(l) = kern_row[l - lag0] for l in [lag0, lag0+len), else 0."""
    n = len(kern_row)

    def w(L):
        Lc = np.clip(L - lag0, 0, n - 1)
        v = kern_row[Lc]
        return np.where((L >= lag0) & (L < lag0 + n), v, 0.0)

    return w


def _hilo(a):
    hi = a.astype(BF16_NP)
    lo = (a - hi.astype(np.float32)).astype(BF16_NP)
    return hi, lo


# ---------------- launch A: projection + conv (8 cores) ----------------

def _build_a():
    nc = bacc.Bacc("TRN2", target_bir_lowering=False, debug=False,
                   num_devices=NCORES)
    se = nc.dram_tensor("se_t", [E_NO, TLOC], FP8, kind="ExternalInput").ap()
    si = nc.dram_tensor("si_t", [I_NO, TLOC], FP8, kind="ExternalInput").ap()
    ce = nc.dram_tensor("ce_t", [E_NO, SUB], FP8, kind="ExternalInput").ap()
    ci = nc.dram_tensor("ci_t", [I_NO, SUB], FP8, kind="ExternalInput").ap()
    # Toeplitz conv weights, bf16 hi/lo: [src e/i][hi/lo][sub, d, CH, CH]
    tk = {}
    for src in "ei":
        for part in ("h", "l"):
            nd = 3 if part == "h" else 2
            tk[(src, part)] = nc.dram_tensor(
                f"tk{src}{part}", [CH, SUB * nd * CH], BF16,
                kind="ExternalInput").ap()
    shard = nc.dram_tensor("shard", [CH, SUB * CPC], F32,
                           kind="ExternalOutput").ap()

    with tile.TileContext(nc) as tc, ExitStack() as ctx:
        const = ctx.enter_context(tc.tile_pool(name="const", bufs=1))
        data = ctx.enter_context(tc.tile_pool(name="data", bufs=1))
        work = ctx.enter_context(tc.tile_pool(name="work", bufs=1))
        ps = ctx.enter_context(tc.tile_pool(name="ps", bufs=4, space="PSUM"))
        psi_pool = ctx.enter_context(
            tc.tile_pool(name="psi", bufs=3, space="PSUM"))
        pso_pool = ctx.enter_context(
            tc.tile_pool(name="pso", bufs=1, space="PSUM"))

        # projection rhs (tiny) loads first -- the first matmuls need it
        def load_proj(tag, ap, n_rows, eng):
            nfull = n_rows // CH
            rem = n_rows - nfull * CH
            wide = const.tile([CH, nfull * SUB], FP8, tag=tag)
            srcap = ap[:nfull * CH, :].rearrange("(a p) s -> p a s", p=CH)
            eng.dma_start(
                wide[:].rearrange("p (a s) -> p a s", a=nfull, s=SUB), srcap)
            out = [wide[:, i * SUB:(i + 1) * SUB] for i in range(nfull)]
            if rem:
                last = const.tile([rem, SUB], FP8, tag=tag + "_l")
                eng.dma_start(last[:], ap[nfull * CH:, :])
                out.append(last[:])
            return out

        ce_tiles = load_proj("ceb", ce, E_NO, nc.sync)
        ci_tiles = load_proj("cib", ci, I_NO, nc.scalar)

        # spike data tiles [e-tile x col-group], byte-balanced across the
        # two HWDGE engines; group 0 lands first so projection starts early
        se_tiles, si_tiles = {}, {}
        _qi = [0]

        def load_spikes(gi):
            c0, nch = COL_GROUPS[gi]
            for src, ap, tiles, store in (("e", se, E_TILES, se_tiles),
                                          ("i", si, I_TILES, si_tiles)):
                for ei, (o, n) in enumerate(tiles):
                    tl = data.tile([n, nch * CH], FP8, tag=f"s{src}{ei}_{gi}")
                    eng = nc.sync if _qi[0] % 2 == 0 else nc.scalar
                    eng.dma_start(tl[:], ap[o:o + n, c0 * CH:(c0 + nch) * CH])
                    store[(ei, gi)] = tl
                    _qi[0] += 1

        load_spikes(0)
        load_spikes(1)

        _weng = [0]

        def load_wide(tag, ap, nmat):
            # DRAM already [CH(j), nsub*nmat*CH(i)] contiguous
            t = const.tile([CH, SUB * nmat * CH], BF16, tag=tag)
            eng = nc.sync if _weng[0] % 2 == 0 else nc.scalar
            _weng[0] += 1
            eng.dma_start(t[:], ap)
            return t

        tk_tiles = {}
        for src in "ei":
            for part, nd in (("h", 3), ("l", 2)):
                wide = load_wide(f"tk{src}{part}", tk[(src, part)], nd)
                for s in range(SUB):
                    for d in range(nd):
                        off = (s * nd + d) * CH
                        tk_tiles[(src, part, s, d)] = wide[:, off:off + CH]


        # s-major projected inputs (integer counts -- exact in bf16)
        synE = work.tile([CH, SUB * LOCAL_CHUNKS], BF16, tag="synE")
        synI = work.tile([CH, SUB * LOCAL_CHUNKS], BF16, tag="synI")
        synE3 = synE[:].rearrange("p (s c) -> p s c", s=SUB, c=LOCAL_CHUNKS)
        synI3 = synI[:].rearrange("p (s c) -> p s c", s=SUB, c=LOCAL_CHUNKS)

        for c in range(LOCAL_CHUNKS):
            gi = 0 if c < COL_GROUPS[1][0] else 1
            off = (c - COL_GROUPS[gi][0]) * CH
            pe = ps.tile([CH, SUB], F32, tag="pse")
            for ei in range(len(E_TILES)):
                nc.tensor.matmul(pe[:], se_tiles[(ei, gi)][:, off:off + CH],
                                 ce_tiles[ei], start=(ei == 0),
                                 stop=(ei == len(E_TILES) - 1))
            nc.vector.tensor_copy(synE3[:, :, c], pe[:])
            pi = psi_pool.tile([CH, SUB], F32, tag="psi")
            for ii in range(len(I_TILES)):
                nc.tensor.matmul(pi[:], si_tiles[(ii, gi)][:, off:off + CH],
                                 ci_tiles[ii], start=(ii == 0),
                                 stop=(ii == len(I_TILES) - 1))
            nc.vector.tensor_copy(synI3[:, :, c], pi[:])

        # depthwise causal conv, lags 0..199 (hi+lo weight passes)
        pso = pso_pool.tile([CH, SUB * CPC], F32)  # s-major [s*CPC + c]
        for s in range(SUB):
            sl = pso[:, s * CPC:(s + 1) * CPC]
            mms = []
            for src, syn3 in (("e", synE3), ("i", synI3)):
                for d in range(3):
                    rhs = syn3[:, s, HALO - d:HALO - d + CPC]
                    mms.append((tk_tiles[(src, "h", s, d)], rhs))
                    if d < 2:
                        mms.append((tk_tiles[(src, "l", s, d)], rhs))
            for mi, (w_t, rhs) in enumerate(mms):
                nc.tensor.matmul(sl, w_t, rhs, start=(mi == 0),
                                 stop=(mi == len(mms) - 1))

        out_t = work.tile([CH, SUB * CPC], F32, tag="out")
        nc.vector.tensor_copy(out_t[:], pso[:])
        nc.sync.dma_start(shard[:], out_t[:])

    nc.compile()
    return nc


# ---------------- launch B: Jacobi scan (1 core) ----------------

def _build_b(pairs, w0, vo):
    """pairs: list of (parent, child, weight) for the lag-1 tree coupling."""
    nc = bacc.Bacc("TRN2", target_bir_lowering=False, debug=False,
                   num_devices=1)
    cin = nc.dram_tensor("cin", [CH, SUB * NCHUNK], F32,
                         kind="ExternalInput").ap()
    hk_h = nc.dram_tensor("hk_h", [CH, SUB * 3 * CH], BF16,
                          kind="ExternalInput").ap()
    hk_l = nc.dram_tensor("hk_l", [CH, SUB * 3 * CH], BF16,
                          kind="ExternalInput").ap()
    npairs = max(len(pairs), 1)
    tr_h = nc.dram_tensor("tr_h", [CH, npairs * 2 * CH], BF16,
                          kind="ExternalInput").ap()
    tr_l = nc.dram_tensor("tr_l", [CH, npairs * 2 * CH], BF16,
                          kind="ExternalInput").ap()
    fv = nc.dram_tensor("fv", [CH, NCHUNK], F32, kind="ExternalOutput").ap()

    group_pairs = {g: [] for g in range(len(GROUPS))}
    for pi, (p, chl, w) in enumerate(pairs):
        for g, (s0, ns) in enumerate(GROUPS):
            if s0 <= p < s0 + ns:
                group_pairs[g].append((pi, p - s0, chl))

    with tile.TileContext(nc) as tc, ExitStack() as ctx:
        const = ctx.enter_context(tc.tile_pool(name="const", bufs=1))
        work = ctx.enter_context(tc.tile_pool(name="work", bufs=1))
        ps = ctx.enter_context(tc.tile_pool(name="ps", bufs=7, space="PSUM"))

        c_t = work.tile([CH, SUB * NCHUNK], F32, tag="c")
        nc.sync.dma_start(c_t[:], cin[:])

        def load_wide(tag, ap, nsub, nmat, eng=None):
            t = const.tile([CH, nsub * nmat * CH], BF16, tag=tag)
            (eng or nc.scalar).dma_start(t[:], ap)
            return t

        hk_tiles, tr_tiles = {}, {}
        for part, ap in (("h", hk_h), ("l", hk_l)):
            wide = load_wide(f"hk{part}", ap, SUB, 3)
            for s in range(SUB):
                for d in range(3):
                    off = (s * 3 + d) * CH
                    hk_tiles[(part, s, d)] = wide[:, off:off + CH]
            if part == "h":
                widet = load_wide("trh", tr_h, len(pairs), 2)
                for pi in range(len(pairs)):
                    for d in range(2):
                        off = (pi * 2 + d) * CH
                        tr_tiles[("h", pi, d)] = widet[:, off:off + CH]
        widet = load_wide("trl", tr_l, len(pairs), 2)
        for pi in range(len(pairs)):
            for d in range(2):
                off = (pi * 2 + d) * CH
                tr_tiles[("l", pi, d)] = widet[:, off:off + CH]

        def y_tile(tag, dtype, full=False):
            t = work.tile([CH, SUB * YC], dtype, tag=tag)
            if full:
                nc.vector.memset(t[:], 0.0)
            else:  # only the per-subunit halo columns are ever read unwritten
                t3 = t[:].rearrange("p (s c) -> p s c", s=SUB, c=YC)
                nc.vector.memset(t3[:, :, 0:HALO], 0.0)
            return t

        ybA = y_tile("ybA", BF16)
        ybB = y_tile("ybB", BF16)
        y32A = y_tile("y32A", F32)
        y32B = y_tile("y32B", F32)
        yhiA = y_tile("yhiA", BF16)
        yloA = y_tile("yloA", BF16)
        yhiB = y_tile("yhiB", BF16)
        yloB = y_tile("yloB", BF16)

        def sub_ap(yt, s, c0, c1):
            """AP for subunit s, chunk columns [c0, c1) of a y buffer."""
            t3 = yt[:].rearrange("p (s c) -> p s c", s=SUB, c=YC)
            return t3[:, s, c0:c1]

        def emit_iter(hi_src, lo_src, use_wlo, dst, post_group=None):
            """One Jacobi iteration: dst = tanh(c + L @ (hi+lo))."""
            for g, (s0, ns) in enumerate(GROUPS):
                pg = ps.tile([CH, ns * NCHUNK], F32, tag="g")
                nc.vector.tensor_copy(
                    pg[:], c_t[:, s0 * NCHUNK:(s0 + ns) * NCHUNK])
                mms = []

                def add(w_t, rhs, iloc):
                    sl = pg[:, iloc * NCHUNK:(iloc + 1) * NCHUNK]
                    mms.append((sl, w_t, rhs))

                for i, s in enumerate(range(s0, s0 + ns)):
                    for d in range(3):
                        c0, c1 = HALO - d, HALO - d + NCHUNK
                        add(hk_tiles[("h", s, d)], sub_ap(hi_src, s, c0, c1), i)
                        if lo_src is not None:
                            add(hk_tiles[("h", s, d)],
                                sub_ap(lo_src, s, c0, c1), i)
                        if use_wlo:
                            add(hk_tiles[("l", s, d)],
                                sub_ap(hi_src, s, c0, c1), i)
                for pi, iloc, chl in group_pairs[g]:
                    for d in range(2):
                        c0, c1 = HALO - d, HALO - d + NCHUNK
                        add(tr_tiles[("h", pi, d)],
                            sub_ap(hi_src, chl, c0, c1), iloc)
                        if lo_src is not None:
                            add(tr_tiles[("h", pi, d)],
                                sub_ap(lo_src, chl, c0, c1), iloc)
                        if use_wlo:
                            add(tr_tiles[("l", pi, d)],
                                sub_ap(hi_src, chl, c0, c1), iloc)
                for mi, (sl, w_t, rhs) in enumerate(mms):
                    nc.tensor.matmul(sl, w_t, rhs, start=False,
                                     stop=(mi == len(mms) - 1),
                                     skip_group_check=True)
                dst3 = dst[:].rearrange("p (s c) -> p s c", s=SUB, c=YC)
                out_ap = dst3[:, s0:s0 + ns, HALO:]
                pg3 = pg[:].rearrange("p (a b) -> p a b", a=ns, b=NCHUNK)
                nc.scalar.activation(out_ap, pg3,
                                     mybir.ActivationFunctionType.Tanh)
                if post_group is not None:
                    post_group(g, ns)

        # iteration 0: y = tanh(c) -- no matmuls (previous iterate is zero)
        for g, (s0, ns) in enumerate(GROUPS):
            pg = ps.tile([CH, ns * NCHUNK], F32, tag="g")
            nc.vector.tensor_copy(
                pg[:], c_t[:, s0 * NCHUNK:(s0 + ns) * NCHUNK])
            dst3 = ybB[:].rearrange("p (s c) -> p s c", s=SUB, c=YC)
            pg3 = pg[:].rearrange("p (a b) -> p a b", a=ns, b=NCHUNK)
            nc.scalar.activation(dst3[:, s0:s0 + ns, HALO:], pg3,
                                 mybir.ActivationFunctionType.Tanh)

        # phase 1: plain bf16 iterations (iteration 0 above counts as one)
        cur = ybB
        for k in range(1, N_BF):
            src = ybB if k % 2 == 1 else ybA
            dst = ybA if k % 2 == 1 else ybB
            emit_iter(src, None, False, dst)
            cur = dst

        # tail: full-precision iterations (hi/lo weights and iterate)
        hi_src, lo_src = cur, None
        y32 = [y32A, y32B]
        hilo = [(yhiA, yloA), (yhiB, yloB)]
        last32 = None
        for k in range(N_TAIL):
            dst = y32[k % 2]
            hi, lo = hilo[k % 2]

            def derive(g, ns, dst=dst, hi=hi, lo=lo):
                s0 = GROUPS[g][0]
                d3 = dst[:].rearrange("p (s c) -> p s c", s=SUB, c=YC)
                h3 = hi[:].rearrange("p (s c) -> p s c", s=SUB, c=YC)
                l3 = lo[:].rearrange("p (s c) -> p s c", s=SUB, c=YC)
                idx = (slice(None), slice(s0, s0 + ns), slice(HALO, YC))
                nc.vector.tensor_copy(h3[idx], d3[idx])
                nc.vector.tensor_tensor(l3[idx], d3[idx], h3[idx],
                                        mybir.AluOpType.subtract)

            is_last = (k == N_TAIL - 1)
            emit_iter(hi_src, lo_src, True, dst,
                      post_group=None if is_last else derive)
            hi_src, lo_src = hi, lo
            last32 = dst

        f1 = work.tile([CH, NCHUNK], F32, tag="f1")
        nc.scalar.activation(f1[:], sub_ap(last32, 0, HALO, YC),
                             mybir.ActivationFunctionType.Copy,
                             bias=float(vo), scale=float(w0))
        nc.sync.dma_start(fv[:], f1[:])

    nc.compile()
    return nc


# ---------------- the public entry point ----------------

def kernel(S_e, S_i, C_den, C_syn_e, C_syn_i, W_syn, Tau_syn, Delta_syn,
           W_sub, V_o, Theta, hist_weights, hist_basis, temp, test):
    trace = _maybe_trace()

    S_e = np.asarray(S_e, dtype=np.float32)
    S_i = np.asarray(S_i, dtype=np.float32)
    C_den = np.asarray(C_den, dtype=np.float32)
    C_syn_e = np.asarray(C_syn_e, dtype=np.float32)
    C_syn_i = np.asarray(C_syn_i, dtype=np.float32)
    W_syn = np.asarray(W_syn, dtype=np.float32)
    Tau_syn = np.asarray(Tau_syn, dtype=np.float32)
    Delta_syn = np.asarray(Delta_syn, dtype=np.float32)
    W_sub = np.asarray(W_sub, dtype=np.float32)
    V_o = np.asarray(V_o, dtype=np.float32)
    Theta = np.asarray(Theta, dtype=np.float32)
    hist_weights = np.asarray(hist_weights, dtype=np.float32)
    hist_basis = np.asarray(hist_basis, dtype=np.float32)

    # --- host parameter math ---
    ke, ki = _alpha_kernels(W_syn, Tau_syn, Delta_syn)
    hist_kern = (hist_weights @ hist_basis).astype(np.float32)  # [20, 200]
    out_filters = np.vstack((ke, ki, hist_kern[:, ::-1])).astype(np.float32)

    tke = np.zeros((SUB, 3, CH, CH), np.float32)
    tki = np.zeros((SUB, 3, CH, CH), np.float32)
    thk = np.zeros((SUB, 3, CH, CH), np.float32)
    for s in range(SUB):
        tke[s, 0], tke[s, 1], tke[s, 2] = _toeplitz_triple(_lagw(ke[s], 0))
        tki[s, 0], tki[s, 1], tki[s, 2] = _toeplitz_triple(_lagw(ki[s], 0))
        thk[s, 0], thk[s, 1], thk[s, 2] = _toeplitz_triple(
            _lagw(hist_kern[s], 1))
    def _pack(a):  # [S, nd, CH, CH] -> [CH(j), S*nd*CH(i)] contiguous
        return np.ascontiguousarray(
            a.transpose(2, 0, 1, 3).reshape(CH, -1))

    tke_h, tke_l = _hilo(tke)
    tki_h, tki_l = _hilo(tki)
    thk_h, thk_l = _hilo(thk)

    # lag-1 tree coupling: prop[p] += C_den[p, ch] * W_sub[ch] * y_{t-1}[ch]
    pairs = []
    pz, cz = np.nonzero(C_den)
    for p, chl in zip(pz.tolist(), cz.tolist()):
        w = float(C_den[p, chl] * W_sub[chl])
        if w != 0.0:
            pairs.append((p, chl, w))
    npairs = max(len(pairs), 1)
    trp = np.zeros((npairs, 2, CH, CH), np.float32)
    for pi, (p, chl, w) in enumerate(pairs):
        t0, t1, _ = _toeplitz_triple(_lagw(np.array([w], np.float32), 1))
        trp[pi, 0], trp[pi, 1] = t0, t1
    trp_h, trp_l = _hilo(trp)

    # --- shard spike inputs (transposed, bf16, HALO leading zero chunks) ---
    pad = HALO * CH
    seT = np.zeros((E_NO, pad + TPAD), FP8_NP)
    seT[:, pad:pad + T] = S_e.astype(FP8_NP).T
    siT = np.zeros((I_NO, pad + TPAD), FP8_NP)
    siT[:, pad:pad + T] = S_i.astype(FP8_NP).T

    ceT = np.ascontiguousarray(C_syn_e.T.astype(FP8_NP))
    ciT = np.ascontiguousarray(C_syn_i.T.astype(FP8_NP))
    in_maps_a = []
    for k in range(NCORES):
        c0 = k * CPC * CH
        in_maps_a.append({
            "se_t": np.ascontiguousarray(seT[:, c0:c0 + TLOC]),
            "si_t": np.ascontiguousarray(siT[:, c0:c0 + TLOC]),
            "ce_t": ceT, "ci_t": ciT,
            "tkeh": _pack(tke_h), "tkel": _pack(tke_l[:, :2]),
            "tkih": _pack(tki_h), "tkil": _pack(tki_l[:, :2]),
        })

    if "A" not in _KCACHE:
        _KCACHE["A"] = _build_a()
    res_a = run_bass_kernel_spmd(_KCACHE["A"], in_maps_a,
                                 core_ids=list(range(NCORES)), trace=trace)
    LAST_PROFILE["a_ns"] = res_a.exec_time_ns
    LAST_PROFILE["a_trace"] = (res_a.instructions_and_trace or (None, None))[1]

    # --- assemble s-major syn_in [128, s*160 + (core*20+c)] + Theta ---
    shards = np.stack([res_a.results[k]["shard"] for k in range(NCORES)])
    # shards: [core, 128, s*CPC + c] -> [128, s, core, c]
    synin = shards.reshape(NCORES, CH, SUB, CPC).transpose(1, 2, 0, 3)
    synin = synin + Theta[None, :, None, None]
    synin = np.ascontiguousarray(
        synin.reshape(CH, SUB * NCHUNK), dtype=np.float32)

    key_b = ("B", tuple(pairs), N_BF, N_TAIL, float(W_sub[0]), float(V_o[0]))
    if key_b not in _KCACHE:
        _KCACHE[key_b] = _build_b(pairs, float(W_sub[0]), float(V_o[0]))
    in_b = {"cin": synin, "hk_h": _pack(thk_h), "hk_l": _pack(thk_l),
            "tr_h": _pack(trp_h), "tr_l": _pack(trp_l)}
    res_b = run_bass_kernel_spmd(_KCACHE[key_b], [in_b], core_ids=[0],
                                 trace=trace)
    LAST_PROFILE["b_ns"] = res_b.exec_time_ns
    LAST_PROFILE["b_trace"] = (res_b.instructions_and_trace or (None, None))[1]

    fv_cj = res_b.results[0]["fv"]  # [128 j, 160 c]; t = c*128 + j
    fv = np.ascontiguousarray(fv_cj.T).reshape(-1)[:T].astype(np.float32)
    return fv, out_filters, C_syn_e, C_syn_i


# revision 50
# speedup vs baseline: 1.0869x; 1.0869x over previous
"""Trainium2 Bass kernel for nn_Alpha_Cos_GLM.

Pipeline (two NEFF launches):
  Launch A (8 cores, data-parallel over time):
    syn_e = S_e @ C_syn_e.T, syn_i = S_i @ C_syn_i.T   (PE, bf16 -- exact for 0/1 data)
    syn_in = causal_conv(syn_e, ke) + causal_conv(syn_i, ki)
      -- depthwise 200-tap conv as per-subunit Toeplitz matmuls in a
         128-time-chunk layout (3 chunk-shift terms cover lags 0..200).
         Weights are bf16 hi+lo pairs (full fp32 precision, 1-pass bf16 rate).
  Launch B (1 core): the sequential scan
      y_t = tanh(syn_in_t + Theta + hist_conv(y) + tree_prop(y_{t-1}))
    via Jacobi fixed-point iteration over the whole padded sequence
    (contraction ~0.7/iter).  Iteration schedule: N_BF plain-bf16 iterations
    (cheap) followed by N_TAIL full-precision iterations where both the
    weights and the iterate are split into bf16 hi+lo parts (3 matmuls per
    logical matmul).  PSUM is preloaded with syn_in by DVE copies; the
    matmuls accumulate on top; one tanh ACT per PSUM bank.

Everything uses an s-major on-chip layout [subunit, chunk] so every matmul
rhs is a contiguous run.
"""

import os
import numpy as np
import ml_dtypes

import concourse.bass as bass
import concourse.tile as tile
from concourse import bacc, mybir
from concourse.bass_utils import run_bass_kernel_spmd
from contextlib import ExitStack

# ---------------- problem constants (hardcoded shapes) ----------------
T = 20000
E_NO = 2000
I_NO = 500
SUB = 20
T_NO = 200
NCORES = 8

CH = 128                 # time chunk
CPC = 20                 # owned chunks per core
NCHUNK = NCORES * CPC    # 160 global chunks
TPAD = NCHUNK * CH       # 20480
HALO = 2                 # halo chunks (lags up to 200 < 2*128)
LOCAL_CHUNKS = CPC + HALO
TLOC = LOCAL_CHUNKS * CH # 2816
YC = NCHUNK + HALO       # per-subunit columns in the scan y buffers

N_BF = int(os.environ.get("GLM_N_BF", "16"))
N_TAIL = int(os.environ.get("GLM_N_TAIL", "0"))

F32 = mybir.dt.float32
BF16 = mybir.dt.bfloat16
BF16_NP = ml_dtypes.bfloat16
FP8 = mybir.dt.float8e4
FP8_NP = ml_dtypes.float8_e4m3
F16 = mybir.dt.float16
F16_NP = np.float16

E_TILES = [(o, min(128, E_NO - o)) for o in range(0, E_NO, 128)]
I_TILES = [(o, min(128, I_NO - o)) for o in range(0, I_NO, 128)]
COL_GROUPS = [(0, 11), (11, 11)]   # (chunk0, nchunks) DMA groups

# scan subunit groups -> one PSUM bank each (ns*NCHUNK floats <= 512)
GROUPS = [(0, 3), (3, 3), (6, 3), (9, 3), (12, 3), (15, 3), (18, 2)]

LAST_PROFILE = {}
_KCACHE = {}


def _maybe_trace():
    if not os.environ.get("GLM_TRACE"):
        return False
    try:  # enable NTFF profiling under axon; harmless no-op if unavailable
        import sys, types
        if "antenv.axon_hooks" not in sys.modules:
            mod = types.ModuleType("antenv.axon_hooks")
            mod._hook = None
            mod.set_axon_ntff_profile_hook = lambda h: setattr(mod, "_hook", h)
            mod.get_axon_ntff_profile_hook = lambda: mod._hook
            sys.modules["antenv.axon_hooks"] = mod
            import antenv
            antenv.axon_hooks = mod
            from trn_agent_boot.trn_boot import _ntff_profile_via_ctypes
            mod.set_axon_ntff_profile_hook(
                _ntff_profile_via_ctypes("/opt/axon/libaxon_pjrt.so"))
        return True
    except Exception:
        return False


# ---------------- host-side parameter math ----------------

def _alpha_kernels(W_syn, Tau_syn, Delta_syn):
    t = np.arange(T_NO, dtype=np.float32)
    te = np.maximum(t[None, None, :] - Delta_syn[:, :, 0, None], 0.0)
    ti = np.maximum(t[None, None, :] - Delta_syn[:, :, 1, None], 0.0)
    te = te / np.exp(Tau_syn[:, :, 0])[:, :, None]
    ti = ti / np.exp(Tau_syn[:, :, 1])[:, :, None]
    ke = np.sum(te * np.exp(-te) * W_syn[:, :, 0, None], axis=1)
    ki = np.sum(ti * np.exp(-ti) * W_syn[:, :, 1, None], axis=1)
    return ke.astype(np.float32), ki.astype(np.float32)


def _toeplitz_triple(wfun):
    """K_d[j,i] = w(i-j+128*d), d=0,1,2: out chunk c reads chunks c,c-1,c-2."""
    idx = np.arange(CH)
    D = idx[None, :] - idx[:, None]  # i - j
    return (wfun(D).astype(np.float32), wfun(D + CH).astype(np.float32),
            wfun(D + 2 * CH).astype(np.float32))


def _lagw(kern_row, lag0):
    """w(l) = kern_row[l - lag0] for l in [lag0, lag0+len), else 0."""
    n = len(kern_row)

    def w(L):
        Lc = np.clip(L - lag0, 0, n - 1)
        v = kern_row[Lc]
        return np.where((L >= lag0) & (L < lag0 + n), v, 0.0)

    return w


def _hilo(a, dt=BF16_NP):
    hi = a.astype(dt)
    lo = (a - hi.astype(np.float32)).astype(dt)
    return hi, lo


# ---------------- launch A: projection + conv (8 cores) ----------------

def _build_a():
    nc = bacc.Bacc("TRN2", target_bir_lowering=False, debug=False,
                   num_devices=NCORES)
    se = nc.dram_tensor("se_t", [E_NO, TLOC], FP8, kind="ExternalInput").ap()
    si = nc.dram_tensor("si_t", [I_NO, TLOC], FP8, kind="ExternalInput").ap()
    ce = nc.dram_tensor("ce_t", [E_NO, SUB], FP8, kind="ExternalInput").ap()
    ci = nc.dram_tensor("ci_t", [I_NO, SUB], FP8, kind="ExternalInput").ap()
    # Toeplitz conv weights, bf16 hi/lo: [src e/i][hi/lo][sub, d, CH, CH]
    tk = {}
    for src in "ei":
        for part in ("h", "l"):
            nd = 3 if part == "h" else 2
            tk[(src, part)] = nc.dram_tensor(
                f"tk{src}{part}", [CH, SUB * nd * CH], BF16,
                kind="ExternalInput").ap()
    shard = nc.dram_tensor("shard", [CH, SUB * CPC], F32,
                           kind="ExternalOutput").ap()

    with tile.TileContext(nc) as tc, ExitStack() as ctx:
        const = ctx.enter_context(tc.tile_pool(name="const", bufs=1))
        data = ctx.enter_context(tc.tile_pool(name="data", bufs=1))
        work = ctx.enter_context(tc.tile_pool(name="work", bufs=1))
        ps = ctx.enter_context(tc.tile_pool(name="ps", bufs=4, space="PSUM"))
        psi_pool = ctx.enter_context(
            tc.tile_pool(name="psi", bufs=3, space="PSUM"))
        pso_pool = ctx.enter_context(
            tc.tile_pool(name="pso", bufs=1, space="PSUM"))

        # projection rhs (tiny) loads first -- the first matmuls need it
        def load_proj(tag, ap, n_rows, eng):
            nfull = n_rows // CH
            rem = n_rows - nfull * CH
            wide = const.tile([CH, nfull * SUB], FP8, tag=tag)
            srcap = ap[:nfull * CH, :].rearrange("(a p) s -> p a s", p=CH)
            eng.dma_start(
                wide[:].rearrange("p (a s) -> p a s", a=nfull, s=SUB), srcap)
            out = [wide[:, i * SUB:(i + 1) * SUB] for i in range(nfull)]
            if rem:
                last = const.tile([rem, SUB], FP8, tag=tag + "_l")
                eng.dma_start(last[:], ap[nfull * CH:, :])
                out.append(last[:])
            return out

        ce_tiles = load_proj("ceb", ce, E_NO, nc.sync)
        ci_tiles = load_proj("cib", ci, I_NO, nc.scalar)

        # spike data tiles [e-tile x col-group], byte-balanced across the
        # two HWDGE engines; group 0 lands first so projection starts early
        se_tiles, si_tiles = {}, {}
        _qi = [0]

        def load_spikes(gi):
            c0, nch = COL_GROUPS[gi]
            for src, ap, tiles, store in (("e", se, E_TILES, se_tiles),
                                          ("i", si, I_TILES, si_tiles)):
                for ei, (o, n) in enumerate(tiles):
                    tl = data.tile([n, nch * CH], FP8, tag=f"s{src}{ei}_{gi}")
                    eng = nc.sync if _qi[0] % 2 == 0 else nc.scalar
                    eng.dma_start(tl[:], ap[o:o + n, c0 * CH:(c0 + nch) * CH])
                    store[(ei, gi)] = tl
                    _qi[0] += 1

        load_spikes(0)
        load_spikes(1)

        _weng = [0]

        def load_wide(tag, ap, nmat):
            # DRAM already [CH(j), nsub*nmat*CH(i)] contiguous
            t = const.tile([CH, SUB * nmat * CH], BF16, tag=tag)
            eng = nc.sync if _weng[0] % 2 == 0 else nc.scalar
            _weng[0] += 1
            eng.dma_start(t[:], ap)
            return t

        tk_tiles = {}
        for src in "ei":
            for part, nd in (("h", 3), ("l", 2)):
                wide = load_wide(f"tk{src}{part}", tk[(src, part)], nd)
                for s in range(SUB):
                    for d in range(nd):
                        off = (s * nd + d) * CH
                        tk_tiles[(src, part, s, d)] = wide[:, off:off + CH]


        # s-major projected inputs (integer counts -- exact in bf16)
        synE = work.tile([CH, SUB * LOCAL_CHUNKS], BF16, tag="synE")
        synI = work.tile([CH, SUB * LOCAL_CHUNKS], BF16, tag="synI")
        synE3 = synE[:].rearrange("p (s c) -> p s c", s=SUB, c=LOCAL_CHUNKS)
        synI3 = synI[:].rearrange("p (s c) -> p s c", s=SUB, c=LOCAL_CHUNKS)

        for c in range(LOCAL_CHUNKS):
            gi = 0 if c < COL_GROUPS[1][0] else 1
            off = (c - COL_GROUPS[gi][0]) * CH
            pe = ps.tile([CH, SUB], F32, tag="pse")
            for ei in range(len(E_TILES)):
                nc.tensor.matmul(pe[:], se_tiles[(ei, gi)][:, off:off + CH],
                                 ce_tiles[ei], start=(ei == 0),
                                 stop=(ei == len(E_TILES) - 1))
            nc.vector.tensor_copy(synE3[:, :, c], pe[:])
            pi = psi_pool.tile([CH, SUB], F32, tag="psi")
            for ii in range(len(I_TILES)):
                nc.tensor.matmul(pi[:], si_tiles[(ii, gi)][:, off:off + CH],
                                 ci_tiles[ii], start=(ii == 0),
                                 stop=(ii == len(I_TILES) - 1))
            nc.vector.tensor_copy(synI3[:, :, c], pi[:])

        # depthwise causal conv, lags 0..199 (hi+lo weight passes)
        pso = pso_pool.tile([CH, SUB * CPC], F32)  # s-major [s*CPC + c]
        for s in range(SUB):
            sl = pso[:, s * CPC:(s + 1) * CPC]
            mms = []
            for src, syn3 in (("e", synE3), ("i", synI3)):
                for d in range(3):
                    rhs = syn3[:, s, HALO - d:HALO - d + CPC]
                    mms.append((tk_tiles[(src, "h", s, d)], rhs))
                    if d < 2:
                        mms.append((tk_tiles[(src, "l", s, d)], rhs))
            for mi, (w_t, rhs) in enumerate(mms):
                nc.tensor.matmul(sl, w_t, rhs, start=(mi == 0),
                                 stop=(mi == len(mms) - 1))

        out_t = work.tile([CH, SUB * CPC], F32, tag="out")
        nc.vector.tensor_copy(out_t[:], pso[:])
        nc.sync.dma_start(shard[:], out_t[:])

    nc.compile()
    return nc


# ---------------- launch B: Jacobi scan (1 core) ----------------

def _build_b(pairs, w0, vo):
    """pairs: list of (parent, child, weight) for the lag-1 tree coupling."""
    nc = bacc.Bacc("TRN2", target_bir_lowering=False, debug=False,
                   num_devices=1)
    cin = nc.dram_tensor("cin", [CH, SUB * NCHUNK], F32,
                         kind="ExternalInput").ap()
    hk_h = nc.dram_tensor("hk_h", [CH, SUB * 3 * CH], F16,
                          kind="ExternalInput").ap()
    npairs = max(len(pairs), 1)
    tr_h = nc.dram_tensor("tr_h", [CH, npairs * 2 * CH], F16,
                          kind="ExternalInput").ap()
    hk_l = tr_l = None
    if N_TAIL:
        hk_l = nc.dram_tensor("hk_l", [CH, SUB * 3 * CH], F16,
                              kind="ExternalInput").ap()
        tr_l = nc.dram_tensor("tr_l", [CH, npairs * 2 * CH], F16,
                              kind="ExternalInput").ap()
    fv = nc.dram_tensor("fv", [CH, NCHUNK], F32, kind="ExternalOutput").ap()

    group_pairs = {g: [] for g in range(len(GROUPS))}
    for pi, (p, chl, w) in enumerate(pairs):
        for g, (s0, ns) in enumerate(GROUPS):
            if s0 <= p < s0 + ns:
                group_pairs[g].append((pi, p - s0, chl))

    with tile.TileContext(nc) as tc, ExitStack() as ctx:
        const = ctx.enter_context(tc.tile_pool(name="const", bufs=1))
        work = ctx.enter_context(tc.tile_pool(name="work", bufs=1))
        ps = ctx.enter_context(tc.tile_pool(name="ps", bufs=7, space="PSUM"))

        c_t = work.tile([CH, SUB * NCHUNK], F32, tag="c")
        nc.sync.dma_start(c_t[:], cin[:])

        def load_wide(tag, ap, nsub, nmat, eng=None):
            t = const.tile([CH, nsub * nmat * CH], F16, tag=tag)
            (eng or nc.scalar).dma_start(t[:], ap)
            return t

        hk_tiles, tr_tiles = {}, {}
        parts = [("h", hk_h, tr_h)] + ([("l", hk_l, tr_l)] if N_TAIL else [])
        for part, hkap, trap in parts:
            wide = load_wide(f"hk{part}", hkap, SUB, 3)
            for s in range(SUB):
                for d in range(3):
                    off = (s * 3 + d) * CH
                    hk_tiles[(part, s, d)] = wide[:, off:off + CH]
            widet = load_wide(f"tr{part}", trap, len(pairs), 2)
            for pi in range(len(pairs)):
                for d in range(2):
                    off = (pi * 2 + d) * CH
                    tr_tiles[(part, pi, d)] = widet[:, off:off + CH]

        def y_tile(tag, dtype, full=False):
            t = work.tile([CH, SUB * YC], dtype, tag=tag)
            if full:
                nc.vector.memset(t[:], 0.0)
            else:  # only the per-subunit halo columns are ever read unwritten
                t3 = t[:].rearrange("p (s c) -> p s c", s=SUB, c=YC)
                nc.vector.memset(t3[:, :, 0:HALO], 0.0)
            return t

        ybA = y_tile("ybA", F16)
        ybB = y_tile("ybB", F16)
        yfin = y_tile("yfin", F32)
        if N_TAIL:
            y32A = y_tile("y32A", F32)
            y32B = y_tile("y32B", F32)
            yhiA = y_tile("yhiA", F16)
            yloA = y_tile("yloA", F16)
            yhiB = y_tile("yhiB", F16)
            yloB = y_tile("yloB", F16)

        def sub_ap(yt, s, c0, c1):
            """AP for subunit s, chunk columns [c0, c1) of a y buffer."""
            t3 = yt[:].rearrange("p (s c) -> p s c", s=SUB, c=YC)
            return t3[:, s, c0:c1]

        def emit_iter(hi_src, lo_src, use_wlo, dst, post_group=None):
            """One Jacobi iteration: dst = tanh(c + L @ (hi+lo))."""
            for g, (s0, ns) in enumerate(GROUPS):
                pg = ps.tile([CH, ns * NCHUNK], F32, tag="g")
                nc.vector.tensor_copy(
                    pg[:], c_t[:, s0 * NCHUNK:(s0 + ns) * NCHUNK])
                mms = []

                def add(w_t, rhs, iloc):
                    sl = pg[:, iloc * NCHUNK:(iloc + 1) * NCHUNK]
                    mms.append((sl, w_t, rhs))

                for i, s in enumerate(range(s0, s0 + ns)):
                    for d in range(3):
                        c0, c1 = HALO - d, HALO - d + NCHUNK
                        add(hk_tiles[("h", s, d)], sub_ap(hi_src, s, c0, c1), i)
                        if lo_src is not None:
                            add(hk_tiles[("h", s, d)],
                                sub_ap(lo_src, s, c0, c1), i)
                        if use_wlo:
                            add(hk_tiles[("l", s, d)],
                                sub_ap(hi_src, s, c0, c1), i)
                for pi, iloc, chl in group_pairs[g]:
                    for d in range(2):
                        c0, c1 = HALO - d, HALO - d + NCHUNK
                        add(tr_tiles[("h", pi, d)],
                            sub_ap(hi_src, chl, c0, c1), iloc)
                        if lo_src is not None:
                            add(tr_tiles[("h", pi, d)],
                                sub_ap(lo_src, chl, c0, c1), iloc)
                        if use_wlo:
                            add(tr_tiles[("l", pi, d)],
                                sub_ap(hi_src, chl, c0, c1), iloc)
                for mi, (sl, w_t, rhs) in enumerate(mms):
                    nc.tensor.matmul(sl, w_t, rhs, start=False,
                                     stop=(mi == len(mms) - 1),
                                     skip_group_check=True)
                dst3 = dst[:].rearrange("p (s c) -> p s c", s=SUB, c=YC)
                out_ap = dst3[:, s0:s0 + ns, HALO:]
                pg3 = pg[:].rearrange("p (a b) -> p a b", a=ns, b=NCHUNK)
                nc.scalar.activation(out_ap, pg3,
                                     mybir.ActivationFunctionType.Tanh)
                if post_group is not None:
                    post_group(g, ns)

        # iteration 0: y = tanh(c) -- no matmuls (previous iterate is zero)
        for g, (s0, ns) in enumerate(GROUPS):
            pg = ps.tile([CH, ns * NCHUNK], F32, tag="g")
            nc.vector.tensor_copy(
                pg[:], c_t[:, s0 * NCHUNK:(s0 + ns) * NCHUNK])
            dst3 = ybB[:].rearrange("p (s c) -> p s c", s=SUB, c=YC)
            pg3 = pg[:].rearrange("p (a b) -> p a b", a=ns, b=NCHUNK)
            nc.scalar.activation(dst3[:, s0:s0 + ns, HALO:], pg3,
                                 mybir.ActivationFunctionType.Tanh)

        # phase 1: plain fp16 iterations (iteration 0 above counts as one)
        cur = ybB
        for k in range(1, N_BF):
            src = ybB if k % 2 == 1 else ybA
            dst = ybA if k % 2 == 1 else ybB
            if k == N_BF - 1 and not N_TAIL:
                dst = yfin  # full-precision output for the extraction
            emit_iter(src, None, False, dst)
            cur = dst

        # tail: full-precision iterations (hi/lo weights and iterate)
        hi_src, lo_src = cur, None
        last32 = cur
        y32 = [y32A, y32B] if N_TAIL else []
        hilo = [(yhiA, yloA), (yhiB, yloB)] if N_TAIL else []
        for k in range(N_TAIL):
            dst = y32[k % 2]
            hi, lo = hilo[k % 2]

            def derive(g, ns, dst=dst, hi=hi, lo=lo):
                s0 = GROUPS[g][0]
                d3 = dst[:].rearrange("p (s c) -> p s c", s=SUB, c=YC)
                h3 = hi[:].rearrange("p (s c) -> p s c", s=SUB, c=YC)
                l3 = lo[:].rearrange("p (s c) -> p s c", s=SUB, c=YC)
                idx = (slice(None), slice(s0, s0 + ns), slice(HALO, YC))
                nc.vector.tensor_copy(h3[idx], d3[idx])
                nc.vector.tensor_tensor(l3[idx], d3[idx], h3[idx],
                                        mybir.AluOpType.subtract)

            is_last = (k == N_TAIL - 1)
            emit_iter(hi_src, lo_src, True, dst,
                      post_group=None if is_last else derive)
            hi_src, lo_src = hi, lo
            last32 = dst

        f1 = work.tile([CH, NCHUNK], F32, tag="f1")
        nc.scalar.activation(f1[:], sub_ap(last32, 0, HALO, YC),
                             mybir.ActivationFunctionType.Copy,
                             bias=float(vo), scale=float(w0))
        nc.sync.dma_start(fv[:], f1[:])

    nc.compile()
    return nc


# ---------------- the public entry point ----------------

def kernel(S_e, S_i, C_den, C_syn_e, C_syn_i, W_syn, Tau_syn, Delta_syn,
           W_sub, V_o, Theta, hist_weights, hist_basis, temp, test):
    trace = _maybe_trace()

    S_e = np.asarray(S_e, dtype=np.float32)
    S_i = np.asarray(S_i, dtype=np.float32)
    C_den = np.asarray(C_den, dtype=np.float32)
    C_syn_e = np.asarray(C_syn_e, dtype=np.float32)
    C_syn_i = np.asarray(C_syn_i, dtype=np.float32)
    W_syn = np.asarray(W_syn, dtype=np.float32)
    Tau_syn = np.asarray(Tau_syn, dtype=np.float32)
    Delta_syn = np.asarray(Delta_syn, dtype=np.float32)
    W_sub = np.asarray(W_sub, dtype=np.float32)
    V_o = np.asarray(V_o, dtype=np.float32)
    Theta = np.asarray(Theta, dtype=np.float32)
    hist_weights = np.asarray(hist_weights, dtype=np.float32)
    hist_basis = np.asarray(hist_basis, dtype=np.float32)

    # --- host parameter math ---
    ke, ki = _alpha_kernels(W_syn, Tau_syn, Delta_syn)
    hist_kern = (hist_weights @ hist_basis).astype(np.float32)  # [20, 200]
    out_filters = np.vstack((ke, ki, hist_kern[:, ::-1])).astype(np.float32)

    tke = np.zeros((SUB, 3, CH, CH), np.float32)
    tki = np.zeros((SUB, 3, CH, CH), np.float32)
    thk = np.zeros((SUB, 3, CH, CH), np.float32)
    for s in range(SUB):
        tke[s, 0], tke[s, 1], tke[s, 2] = _toeplitz_triple(_lagw(ke[s], 0))
        tki[s, 0], tki[s, 1], tki[s, 2] = _toeplitz_triple(_lagw(ki[s], 0))
        thk[s, 0], thk[s, 1], thk[s, 2] = _toeplitz_triple(
            _lagw(hist_kern[s], 1))
    def _pack(a):  # [S, nd, CH, CH] -> [CH(j), S*nd*CH(i)] contiguous
        return np.ascontiguousarray(
            a.transpose(2, 0, 1, 3).reshape(CH, -1))

    tke_h, tke_l = _hilo(tke)
    tki_h, tki_l = _hilo(tki)
    thk_h, thk_l = _hilo(thk, F16_NP)

    # lag-1 tree coupling: prop[p] += C_den[p, ch] * W_sub[ch] * y_{t-1}[ch]
    pairs = []
    pz, cz = np.nonzero(C_den)
    for p, chl in zip(pz.tolist(), cz.tolist()):
        w = float(C_den[p, chl] * W_sub[chl])
        if w != 0.0:
            pairs.append((p, chl, w))
    npairs = max(len(pairs), 1)
    trp = np.zeros((npairs, 2, CH, CH), np.float32)
    for pi, (p, chl, w) in enumerate(pairs):
        t0, t1, _ = _toeplitz_triple(_lagw(np.array([w], np.float32), 1))
        trp[pi, 0], trp[pi, 1] = t0, t1
    trp_h, trp_l = _hilo(trp, F16_NP)

    # --- shard spike inputs (transposed, bf16, HALO leading zero chunks) ---
    pad = HALO * CH
    seT = np.zeros((E_NO, pad + TPAD), FP8_NP)
    seT[:, pad:pad + T] = S_e.astype(FP8_NP).T
    siT = np.zeros((I_NO, pad + TPAD), FP8_NP)
    siT[:, pad:pad + T] = S_i.astype(FP8_NP).T

    ceT = np.ascontiguousarray(C_syn_e.T.astype(FP8_NP))
    ciT = np.ascontiguousarray(C_syn_i.T.astype(FP8_NP))
    in_maps_a = []
    for k in range(NCORES):
        c0 = k * CPC * CH
        in_maps_a.append({
            "se_t": np.ascontiguousarray(seT[:, c0:c0 + TLOC]),
            "si_t": np.ascontiguousarray(siT[:, c0:c0 + TLOC]),
            "ce_t": ceT, "ci_t": ciT,
            "tkeh": _pack(tke_h), "tkel": _pack(tke_l[:, :2]),
            "tkih": _pack(tki_h), "tkil": _pack(tki_l[:, :2]),
        })

    if "A" not in _KCACHE:
        _KCACHE["A"] = _build_a()
    res_a = run_bass_kernel_spmd(_KCACHE["A"], in_maps_a,
                                 core_ids=list(range(NCORES)), trace=trace)
    LAST_PROFILE["a_ns"] = res_a.exec_time_ns
    LAST_PROFILE["a_trace"] = (res_a.instructions_and_trace or (None, None))[1]

    # --- assemble s-major syn_in [128, s*160 + (core*20+c)] + Theta ---
    shards = np.stack([res_a.results[k]["shard"] for k in range(NCORES)])
    # shards: [core, 128, s*CPC + c] -> [128, s, core, c]
    synin = shards.reshape(NCORES, CH, SUB, CPC).transpose(1, 2, 0, 3)
    synin = synin + Theta[None, :, None, None]
    synin = np.ascontiguousarray(
        synin.reshape(CH, SUB * NCHUNK), dtype=np.float32)

    key_b = ("B", tuple(pairs), N_BF, N_TAIL, float(W_sub[0]), float(V_o[0]))
    if key_b not in _KCACHE:
        _KCACHE[key_b] = _build_b(pairs, float(W_sub[0]), float(V_o[0]))
    in_b = {"cin": synin, "hk_h": _pack(thk_h), "tr_h": _pack(trp_h)}
    if N_TAIL:
        in_b["hk_l"] = _pack(thk_l)
        in_b["tr_l"] = _pack(trp_l)
    res_b = run_bass_kernel_spmd(_KCACHE[key_b], [in_b], core_ids=[0],
                                 trace=trace)
    LAST_PROFILE["b_ns"] = res_b.exec_time_ns
    LAST_PROFILE["b_trace"] = (res_b.instructions_and_trace or (None, None))[1]

    fv_cj = res_b.results[0]["fv"]  # [128 j, 160 c]; t = c*128 + j
    fv = np.ascontiguousarray(fv_cj.T).reshape(-1)[:T].astype(np.float32)
    return fv, out_filters, C_syn_e, C_syn_i


# revision 51
# speedup vs baseline: 1.2041x; 1.1079x over previous
"""Trainium2 Bass kernel for nn_Alpha_Cos_GLM.

Pipeline (two NEFF launches):
  Launch A (8 cores, data-parallel over time):
    syn_e = S_e @ C_syn_e.T, syn_i = S_i @ C_syn_i.T   (PE, bf16 -- exact for 0/1 data)
    syn_in = causal_conv(syn_e, ke) + causal_conv(syn_i, ki)
      -- depthwise 200-tap conv as per-subunit Toeplitz matmuls in a
         128-time-chunk layout (3 chunk-shift terms cover lags 0..200).
         Weights are bf16 hi+lo pairs (full fp32 precision, 1-pass bf16 rate).
  Launch B (1 core): the sequential scan
      y_t = tanh(syn_in_t + Theta + hist_conv(y) + tree_prop(y_{t-1}))
    via Jacobi fixed-point iteration over the whole padded sequence
    (contraction ~0.7/iter).  Iteration schedule: N_BF plain-bf16 iterations
    (cheap) followed by N_TAIL full-precision iterations where both the
    weights and the iterate are split into bf16 hi+lo parts (3 matmuls per
    logical matmul).  PSUM is preloaded with syn_in by DVE copies; the
    matmuls accumulate on top; one tanh ACT per PSUM bank.

Everything uses an s-major on-chip layout [subunit, chunk] so every matmul
rhs is a contiguous run.
"""

import os
import numpy as np
import ml_dtypes

import concourse.bass as bass
import concourse.tile as tile
from concourse import bacc, mybir
from concourse.bass_utils import run_bass_kernel_spmd
from contextlib import ExitStack

# ---------------- problem constants (hardcoded shapes) ----------------
T = 20000
E_NO = 2000
I_NO = 500
SUB = 20
T_NO = 200
NCORES = 8

CH = 128                 # time chunk
CPC = 20                 # owned chunks per core
NCHUNK = NCORES * CPC    # 160 global chunks
TPAD = NCHUNK * CH       # 20480
HALO = 2                 # halo chunks (lags up to 200 < 2*128)
LOCAL_CHUNKS = CPC + HALO
TLOC = LOCAL_CHUNKS * CH # 2816
YC = NCHUNK + HALO       # per-subunit columns in the scan y buffers

N_BF = int(os.environ.get("GLM_N_BF", "16"))
N_TAIL = int(os.environ.get("GLM_N_TAIL", "0"))

F32 = mybir.dt.float32
BF16 = mybir.dt.bfloat16
BF16_NP = ml_dtypes.bfloat16
FP8 = mybir.dt.float8e4
FP8_NP = ml_dtypes.float8_e4m3
F16 = mybir.dt.float16
F16_NP = np.float16

E_TILES = [(o, min(128, E_NO - o)) for o in range(0, E_NO, 128)]
I_TILES = [(o, min(128, I_NO - o)) for o in range(0, I_NO, 128)]
COL_GROUPS = [(0, 11), (11, 11)]   # (chunk0, nchunks) DMA groups

# scan subunit groups -> one PSUM bank each (ns*NCHUNK floats <= 512)
GROUPS = [(0, 3), (3, 3), (6, 3), (9, 3), (12, 3), (15, 3), (18, 2)]

LAST_PROFILE = {}
_KCACHE = {}


def _maybe_trace():
    if not os.environ.get("GLM_TRACE"):
        return False
    try:  # enable NTFF profiling under axon; harmless no-op if unavailable
        import sys, types
        if "antenv.axon_hooks" not in sys.modules:
            mod = types.ModuleType("antenv.axon_hooks")
            mod._hook = None
            mod.set_axon_ntff_profile_hook = lambda h: setattr(mod, "_hook", h)
            mod.get_axon_ntff_profile_hook = lambda: mod._hook
            sys.modules["antenv.axon_hooks"] = mod
            import antenv
            antenv.axon_hooks = mod
            from trn_agent_boot.trn_boot import _ntff_profile_via_ctypes
            mod.set_axon_ntff_profile_hook(
                _ntff_profile_via_ctypes("/opt/axon/libaxon_pjrt.so"))
        return True
    except Exception:
        return False


# ---------------- host-side parameter math ----------------

def _alpha_kernels(W_syn, Tau_syn, Delta_syn):
    t = np.arange(T_NO, dtype=np.float32)
    te = np.maximum(t[None, None, :] - Delta_syn[:, :, 0, None], 0.0)
    ti = np.maximum(t[None, None, :] - Delta_syn[:, :, 1, None], 0.0)
    te = te / np.exp(Tau_syn[:, :, 0])[:, :, None]
    ti = ti / np.exp(Tau_syn[:, :, 1])[:, :, None]
    ke = np.sum(te * np.exp(-te) * W_syn[:, :, 0, None], axis=1)
    ki = np.sum(ti * np.exp(-ti) * W_syn[:, :, 1, None], axis=1)
    return ke.astype(np.float32), ki.astype(np.float32)


def _toeplitz_triple(wfun):
    """K_d[j,i] = w(i-j+128*d), d=0,1,2: out chunk c reads chunks c,c-1,c-2."""
    idx = np.arange(CH)
    D = idx[None, :] - idx[:, None]  # i - j
    return (wfun(D).astype(np.float32), wfun(D + CH).astype(np.float32),
            wfun(D + 2 * CH).astype(np.float32))


def _lagw(kern_row, lag0):
    """w(l) = kern_row[l - lag0] for l in [lag0, lag0+len), else 0."""
    n = len(kern_row)

    def w(L):
        Lc = np.clip(L - lag0, 0, n - 1)
        v = kern_row[Lc]
        return np.where((L >= lag0) & (L < lag0 + n), v, 0.0)

    return w


def _hilo(a, dt=BF16_NP):
    hi = a.astype(dt)
    lo = (a - hi.astype(np.float32)).astype(dt)
    return hi, lo


# ---------------- launch A: projection + conv (8 cores) ----------------

def _build_a():
    nc = bacc.Bacc("TRN2", target_bir_lowering=False, debug=False,
                   num_devices=NCORES)
    se = nc.dram_tensor("se_t", [E_NO, TLOC], FP8, kind="ExternalInput").ap()
    si = nc.dram_tensor("si_t", [I_NO, TLOC], FP8, kind="ExternalInput").ap()
    ce = nc.dram_tensor("ce_t", [E_NO, SUB], FP8, kind="ExternalInput").ap()
    ci = nc.dram_tensor("ci_t", [I_NO, SUB], FP8, kind="ExternalInput").ap()
    # Toeplitz conv weights, single fp16 (11-bit mantissa suffices)
    tk = {}
    for src in "ei":
        tk[src] = nc.dram_tensor(
            f"tk{src}", [CH, SUB * 3 * CH], F16, kind="ExternalInput").ap()
    shard = nc.dram_tensor("shard", [CH, SUB * CPC], F32,
                           kind="ExternalOutput").ap()

    with tile.TileContext(nc) as tc, ExitStack() as ctx:
        const = ctx.enter_context(tc.tile_pool(name="const", bufs=1))
        data = ctx.enter_context(tc.tile_pool(name="data", bufs=1))
        work = ctx.enter_context(tc.tile_pool(name="work", bufs=1))
        ps = ctx.enter_context(tc.tile_pool(name="ps", bufs=4, space="PSUM"))
        psi_pool = ctx.enter_context(
            tc.tile_pool(name="psi", bufs=3, space="PSUM"))
        pso_pool = ctx.enter_context(
            tc.tile_pool(name="pso", bufs=1, space="PSUM"))

        # projection rhs (tiny) loads first -- the first matmuls need it
        def load_proj(tag, ap, n_rows, eng):
            nfull = n_rows // CH
            rem = n_rows - nfull * CH
            wide = const.tile([CH, nfull * SUB], FP8, tag=tag)
            srcap = ap[:nfull * CH, :].rearrange("(a p) s -> p a s", p=CH)
            eng.dma_start(
                wide[:].rearrange("p (a s) -> p a s", a=nfull, s=SUB), srcap)
            out = [wide[:, i * SUB:(i + 1) * SUB] for i in range(nfull)]
            if rem:
                last = const.tile([rem, SUB], FP8, tag=tag + "_l")
                eng.dma_start(last[:], ap[nfull * CH:, :])
                out.append(last[:])
            return out

        ce_tiles = load_proj("ceb", ce, E_NO, nc.sync)
        ci_tiles = load_proj("cib", ci, I_NO, nc.scalar)

        # spike data tiles [e-tile x col-group], byte-balanced across the
        # two HWDGE engines; group 0 lands first so projection starts early
        se_tiles, si_tiles = {}, {}
        _qi = [0]

        def load_spikes(gi):
            c0, nch = COL_GROUPS[gi]
            for src, ap, tiles, store in (("e", se, E_TILES, se_tiles),
                                          ("i", si, I_TILES, si_tiles)):
                for ei, (o, n) in enumerate(tiles):
                    tl = data.tile([n, nch * CH], FP8, tag=f"s{src}{ei}_{gi}")
                    eng = nc.sync if _qi[0] % 2 == 0 else nc.scalar
                    eng.dma_start(tl[:], ap[o:o + n, c0 * CH:(c0 + nch) * CH])
                    store[(ei, gi)] = tl
                    _qi[0] += 1

        load_spikes(0)
        load_spikes(1)

        tk_tiles = {}
        for qi2, src in enumerate("ei"):
            t = const.tile([CH, SUB * 3 * CH], F16, tag=f"tk{src}")
            eng = nc.sync if qi2 % 2 == 0 else nc.scalar
            eng.dma_start(t[:], tk[src])
            for s in range(SUB):
                for d in range(3):
                    off = (s * 3 + d) * CH
                    tk_tiles[(src, s, d)] = t[:, off:off + CH]


        # s-major projected inputs (integer counts -- exact in bf16)
        synE = work.tile([CH, SUB * LOCAL_CHUNKS], F16, tag="synE")
        synI = work.tile([CH, SUB * LOCAL_CHUNKS], F16, tag="synI")
        synE3 = synE[:].rearrange("p (s c) -> p s c", s=SUB, c=LOCAL_CHUNKS)
        synI3 = synI[:].rearrange("p (s c) -> p s c", s=SUB, c=LOCAL_CHUNKS)

        for c in range(LOCAL_CHUNKS):
            gi = 0 if c < COL_GROUPS[1][0] else 1
            off = (c - COL_GROUPS[gi][0]) * CH
            pe = ps.tile([CH, SUB], F32, tag="pse")
            for ei in range(len(E_TILES)):
                nc.tensor.matmul(pe[:], se_tiles[(ei, gi)][:, off:off + CH],
                                 ce_tiles[ei], start=(ei == 0),
                                 stop=(ei == len(E_TILES) - 1))
            nc.vector.tensor_copy(synE3[:, :, c], pe[:])
            pi = psi_pool.tile([CH, SUB], F32, tag="psi")
            for ii in range(len(I_TILES)):
                nc.tensor.matmul(pi[:], si_tiles[(ii, gi)][:, off:off + CH],
                                 ci_tiles[ii], start=(ii == 0),
                                 stop=(ii == len(I_TILES) - 1))
            nc.vector.tensor_copy(synI3[:, :, c], pi[:])

        # depthwise causal conv, lags 0..199 (hi+lo weight passes)
        pso = pso_pool.tile([CH, SUB * CPC], F32)  # s-major [s*CPC + c]
        for s in range(SUB):
            sl = pso[:, s * CPC:(s + 1) * CPC]
            mms = []
            for src, syn3 in (("e", synE3), ("i", synI3)):
                for d in range(3):
                    rhs = syn3[:, s, HALO - d:HALO - d + CPC]
                    mms.append((tk_tiles[(src, s, d)], rhs))
            for mi, (w_t, rhs) in enumerate(mms):
                nc.tensor.matmul(sl, w_t, rhs, start=(mi == 0),
                                 stop=(mi == len(mms) - 1))

        out_t = work.tile([CH, SUB * CPC], F32, tag="out")
        nc.vector.tensor_copy(out_t[:], pso[:])
        nc.sync.dma_start(shard[:], out_t[:])

    nc.compile()
    return nc


# ---------------- launch B: Jacobi scan (1 core) ----------------

def _build_b(pairs, w0, vo):
    """pairs: list of (parent, child, weight) for the lag-1 tree coupling."""
    nc = bacc.Bacc("TRN2", target_bir_lowering=False, debug=False,
                   num_devices=1)
    cin = nc.dram_tensor("cin", [CH, SUB * NCHUNK], F32,
                         kind="ExternalInput").ap()
    hk_h = nc.dram_tensor("hk_h", [CH, SUB * 3 * CH], F16,
                          kind="ExternalInput").ap()
    npairs = max(len(pairs), 1)
    tr_h = nc.dram_tensor("tr_h", [CH, npairs * 2 * CH], F16,
                          kind="ExternalInput").ap()
    hk_l = tr_l = None
    if N_TAIL:
        hk_l = nc.dram_tensor("hk_l", [CH, SUB * 3 * CH], F16,
                              kind="ExternalInput").ap()
        tr_l = nc.dram_tensor("tr_l", [CH, npairs * 2 * CH], F16,
                              kind="ExternalInput").ap()
    fv = nc.dram_tensor("fv", [CH, NCHUNK], F32, kind="ExternalOutput").ap()

    group_pairs = {g: [] for g in range(len(GROUPS))}
    for pi, (p, chl, w) in enumerate(pairs):
        for g, (s0, ns) in enumerate(GROUPS):
            if s0 <= p < s0 + ns:
                group_pairs[g].append((pi, p - s0, chl))

    with tile.TileContext(nc) as tc, ExitStack() as ctx:
        const = ctx.enter_context(tc.tile_pool(name="const", bufs=1))
        work = ctx.enter_context(tc.tile_pool(name="work", bufs=1))
        ps = ctx.enter_context(tc.tile_pool(name="ps", bufs=7, space="PSUM"))

        c_t = work.tile([CH, SUB * NCHUNK], F32, tag="c")
        nc.sync.dma_start(c_t[:], cin[:])

        def load_wide(tag, ap, nsub, nmat, eng=None):
            t = const.tile([CH, nsub * nmat * CH], F16, tag=tag)
            (eng or nc.scalar).dma_start(t[:], ap)
            return t

        hk_tiles, tr_tiles = {}, {}
        parts = [("h", hk_h, tr_h)] + ([("l", hk_l, tr_l)] if N_TAIL else [])
        for part, hkap, trap in parts:
            wide = load_wide(f"hk{part}", hkap, SUB, 3)
            for s in range(SUB):
                for d in range(3):
                    off = (s * 3 + d) * CH
                    hk_tiles[(part, s, d)] = wide[:, off:off + CH]
            widet = load_wide(f"tr{part}", trap, len(pairs), 2)
            for pi in range(len(pairs)):
                for d in range(2):
                    off = (pi * 2 + d) * CH
                    tr_tiles[(part, pi, d)] = widet[:, off:off + CH]

        def y_tile(tag, dtype, full=False):
            t = work.tile([CH, SUB * YC], dtype, tag=tag)
            if full:
                nc.vector.memset(t[:], 0.0)
            else:  # only the per-subunit halo columns are ever read unwritten
                t3 = t[:].rearrange("p (s c) -> p s c", s=SUB, c=YC)
                nc.vector.memset(t3[:, :, 0:HALO], 0.0)
            return t

        ybA = y_tile("ybA", F16)
        ybB = y_tile("ybB", F16)
        yfin = y_tile("yfin", F32)
        if N_TAIL:
            y32A = y_tile("y32A", F32)
            y32B = y_tile("y32B", F32)
            yhiA = y_tile("yhiA", F16)
            yloA = y_tile("yloA", F16)
            yhiB = y_tile("yhiB", F16)
            yloB = y_tile("yloB", F16)

        def sub_ap(yt, s, c0, c1):
            """AP for subunit s, chunk columns [c0, c1) of a y buffer."""
            t3 = yt[:].rearrange("p (s c) -> p s c", s=SUB, c=YC)
            return t3[:, s, c0:c1]

        def emit_iter(hi_src, lo_src, use_wlo, dst, post_group=None):
            """One Jacobi iteration: dst = tanh(c + L @ (hi+lo))."""
            for g, (s0, ns) in enumerate(GROUPS):
                pg = ps.tile([CH, ns * NCHUNK], F32, tag="g")
                nc.vector.tensor_copy(
                    pg[:], c_t[:, s0 * NCHUNK:(s0 + ns) * NCHUNK])
                mms = []

                def add(w_t, rhs, iloc):
                    sl = pg[:, iloc * NCHUNK:(iloc + 1) * NCHUNK]
                    mms.append((sl, w_t, rhs))

                for i, s in enumerate(range(s0, s0 + ns)):
                    for d in range(3):
                        c0, c1 = HALO - d, HALO - d + NCHUNK
                        add(hk_tiles[("h", s, d)], sub_ap(hi_src, s, c0, c1), i)
                        if lo_src is not None:
                            add(hk_tiles[("h", s, d)],
                                sub_ap(lo_src, s, c0, c1), i)
                        if use_wlo:
                            add(hk_tiles[("l", s, d)],
                                sub_ap(hi_src, s, c0, c1), i)
                for pi, iloc, chl in group_pairs[g]:
                    for d in range(2):
                        c0, c1 = HALO - d, HALO - d + NCHUNK
                        add(tr_tiles[("h", pi, d)],
                            sub_ap(hi_src, chl, c0, c1), iloc)
                        if lo_src is not None:
                            add(tr_tiles[("h", pi, d)],
                                sub_ap(lo_src, chl, c0, c1), iloc)
                        if use_wlo:
                            add(tr_tiles[("l", pi, d)],
                                sub_ap(hi_src, chl, c0, c1), iloc)
                for mi, (sl, w_t, rhs) in enumerate(mms):
                    nc.tensor.matmul(sl, w_t, rhs, start=False,
                                     stop=(mi == len(mms) - 1),
                                     skip_group_check=True)
                dst3 = dst[:].rearrange("p (s c) -> p s c", s=SUB, c=YC)
                out_ap = dst3[:, s0:s0 + ns, HALO:]
                pg3 = pg[:].rearrange("p (a b) -> p a b", a=ns, b=NCHUNK)
                nc.scalar.activation(out_ap, pg3,
                                     mybir.ActivationFunctionType.Tanh)
                if post_group is not None:
                    post_group(g, ns)

        # iteration 0: y = tanh(c) -- no matmuls (previous iterate is zero)
        for g, (s0, ns) in enumerate(GROUPS):
            pg = ps.tile([CH, ns * NCHUNK], F32, tag="g")
            nc.vector.tensor_copy(
                pg[:], c_t[:, s0 * NCHUNK:(s0 + ns) * NCHUNK])
            dst3 = ybB[:].rearrange("p (s c) -> p s c", s=SUB, c=YC)
            pg3 = pg[:].rearrange("p (a b) -> p a b", a=ns, b=NCHUNK)
            nc.scalar.activation(dst3[:, s0:s0 + ns, HALO:], pg3,
                                 mybir.ActivationFunctionType.Tanh)

        # phase 1: plain fp16 iterations (iteration 0 above counts as one)
        cur = ybB
        for k in range(1, N_BF):
            src = ybB if k % 2 == 1 else ybA
            dst = ybA if k % 2 == 1 else ybB
            if k == N_BF - 1 and not N_TAIL:
                dst = yfin  # full-precision output for the extraction
            emit_iter(src, None, False, dst)
            cur = dst

        # tail: full-precision iterations (hi/lo weights and iterate)
        hi_src, lo_src = cur, None
        last32 = cur
        y32 = [y32A, y32B] if N_TAIL else []
        hilo = [(yhiA, yloA), (yhiB, yloB)] if N_TAIL else []
        for k in range(N_TAIL):
            dst = y32[k % 2]
            hi, lo = hilo[k % 2]

            def derive(g, ns, dst=dst, hi=hi, lo=lo):
                s0 = GROUPS[g][0]
                d3 = dst[:].rearrange("p (s c) -> p s c", s=SUB, c=YC)
                h3 = hi[:].rearrange("p (s c) -> p s c", s=SUB, c=YC)
                l3 = lo[:].rearrange("p (s c) -> p s c", s=SUB, c=YC)
                idx = (slice(None), slice(s0, s0 + ns), slice(HALO, YC))
                nc.vector.tensor_copy(h3[idx], d3[idx])
                nc.vector.tensor_tensor(l3[idx], d3[idx], h3[idx],
                                        mybir.AluOpType.subtract)

            is_last = (k == N_TAIL - 1)
            emit_iter(hi_src, lo_src, True, dst,
                      post_group=None if is_last else derive)
            hi_src, lo_src = hi, lo
            last32 = dst

        f1 = work.tile([CH, NCHUNK], F32, tag="f1")
        nc.scalar.activation(f1[:], sub_ap(last32, 0, HALO, YC),
                             mybir.ActivationFunctionType.Copy,
                             bias=float(vo), scale=float(w0))
        nc.sync.dma_start(fv[:], f1[:])

    nc.compile()
    return nc


# ---------------- the public entry point ----------------

def kernel(S_e, S_i, C_den, C_syn_e, C_syn_i, W_syn, Tau_syn, Delta_syn,
           W_sub, V_o, Theta, hist_weights, hist_basis, temp, test):
    trace = _maybe_trace()

    S_e = np.asarray(S_e, dtype=np.float32)
    S_i = np.asarray(S_i, dtype=np.float32)
    C_den = np.asarray(C_den, dtype=np.float32)
    C_syn_e = np.asarray(C_syn_e, dtype=np.float32)
    C_syn_i = np.asarray(C_syn_i, dtype=np.float32)
    W_syn = np.asarray(W_syn, dtype=np.float32)
    Tau_syn = np.asarray(Tau_syn, dtype=np.float32)
    Delta_syn = np.asarray(Delta_syn, dtype=np.float32)
    W_sub = np.asarray(W_sub, dtype=np.float32)
    V_o = np.asarray(V_o, dtype=np.float32)
    Theta = np.asarray(Theta, dtype=np.float32)
    hist_weights = np.asarray(hist_weights, dtype=np.float32)
    hist_basis = np.asarray(hist_basis, dtype=np.float32)

    # --- host parameter math ---
    ke, ki = _alpha_kernels(W_syn, Tau_syn, Delta_syn)
    hist_kern = (hist_weights @ hist_basis).astype(np.float32)  # [20, 200]
    out_filters = np.vstack((ke, ki, hist_kern[:, ::-1])).astype(np.float32)

    tke = np.zeros((SUB, 3, CH, CH), np.float32)
    tki = np.zeros((SUB, 3, CH, CH), np.float32)
    thk = np.zeros((SUB, 3, CH, CH), np.float32)
    for s in range(SUB):
        tke[s, 0], tke[s, 1], tke[s, 2] = _toeplitz_triple(_lagw(ke[s], 0))
        tki[s, 0], tki[s, 1], tki[s, 2] = _toeplitz_triple(_lagw(ki[s], 0))
        thk[s, 0], thk[s, 1], thk[s, 2] = _toeplitz_triple(
            _lagw(hist_kern[s], 1))
    def _pack(a):  # [S, nd, CH, CH] -> [CH(j), S*nd*CH(i)] contiguous
        return np.ascontiguousarray(
            a.transpose(2, 0, 1, 3).reshape(CH, -1))

    thk_h, thk_l = _hilo(thk, F16_NP)

    # lag-1 tree coupling: prop[p] += C_den[p, ch] * W_sub[ch] * y_{t-1}[ch]
    pairs = []
    pz, cz = np.nonzero(C_den)
    for p, chl in zip(pz.tolist(), cz.tolist()):
        w = float(C_den[p, chl] * W_sub[chl])
        if w != 0.0:
            pairs.append((p, chl, w))
    npairs = max(len(pairs), 1)
    trp = np.zeros((npairs, 2, CH, CH), np.float32)
    for pi, (p, chl, w) in enumerate(pairs):
        t0, t1, _ = _toeplitz_triple(_lagw(np.array([w], np.float32), 1))
        trp[pi, 0], trp[pi, 1] = t0, t1
    trp_h, trp_l = _hilo(trp, F16_NP)

    # --- shard spike inputs (transposed, bf16, HALO leading zero chunks) ---
    pad = HALO * CH
    seT = np.zeros((E_NO, pad + TPAD), FP8_NP)
    seT[:, pad:pad + T] = S_e.astype(FP8_NP).T
    siT = np.zeros((I_NO, pad + TPAD), FP8_NP)
    siT[:, pad:pad + T] = S_i.astype(FP8_NP).T

    ceT = np.ascontiguousarray(C_syn_e.T.astype(FP8_NP))
    ciT = np.ascontiguousarray(C_syn_i.T.astype(FP8_NP))
    in_maps_a = []
    for k in range(NCORES):
        c0 = k * CPC * CH
        in_maps_a.append({
            "se_t": np.ascontiguousarray(seT[:, c0:c0 + TLOC]),
            "si_t": np.ascontiguousarray(siT[:, c0:c0 + TLOC]),
            "ce_t": ceT, "ci_t": ciT,
            "tke": _pack(tke.astype(F16_NP)),
            "tki": _pack(tki.astype(F16_NP)),
        })

    if "A" not in _KCACHE:
        _KCACHE["A"] = _build_a()
    res_a = run_bass_kernel_spmd(_KCACHE["A"], in_maps_a,
                                 core_ids=list(range(NCORES)), trace=trace)
    LAST_PROFILE["a_ns"] = res_a.exec_time_ns
    LAST_PROFILE["a_trace"] = (res_a.instructions_and_trace or (None, None))[1]

    # --- assemble s-major syn_in [128, s*160 + (core*20+c)] + Theta ---
    shards = np.stack([res_a.results[k]["shard"] for k in range(NCORES)])
    # shards: [core, 128, s*CPC + c] -> [128, s, core, c]
    synin = shards.reshape(NCORES, CH, SUB, CPC).transpose(1, 2, 0, 3)
    synin = synin + Theta[None, :, None, None]
    synin = np.ascontiguousarray(
        synin.reshape(CH, SUB * NCHUNK), dtype=np.float32)

    key_b = ("B", tuple(pairs), N_BF, N_TAIL, float(W_sub[0]), float(V_o[0]))
    if key_b not in _KCACHE:
        _KCACHE[key_b] = _build_b(pairs, float(W_sub[0]), float(V_o[0]))
    in_b = {"cin": synin, "hk_h": _pack(thk_h), "tr_h": _pack(trp_h)}
    if N_TAIL:
        in_b["hk_l"] = _pack(thk_l)
        in_b["tr_l"] = _pack(trp_l)
    res_b = run_bass_kernel_spmd(_KCACHE[key_b], [in_b], core_ids=[0],
                                 trace=trace)
    LAST_PROFILE["b_ns"] = res_b.exec_time_ns
    LAST_PROFILE["b_trace"] = (res_b.instructions_and_trace or (None, None))[1]

    fv_cj = res_b.results[0]["fv"]  # [128 j, 160 c]; t = c*128 + j
    fv = np.ascontiguousarray(fv_cj.T).reshape(-1)[:T].astype(np.float32)
    return fv, out_filters, C_syn_e, C_syn_i
